# revision 24
# baseline (speedup 1.0000x reference)
"""Bass/Trainium2 kernel for nn_KernelAMController (retrieval_knn).

Math: out(b,:) = -sum_g w(b,g)*mask[tb,g]*adj[tb(b),g,:] / (sum_g w*mask + eps)
with w(b,g) = exp(-2*||x_b - p_g||^2).

The Gaussian kernel (bandwidth 0.5) is spatially local: grid points beyond
~1.4 units contribute ~4e-3 relative error (tolerance 2e-2). Samples are
k-d sorted (host) into 64 leaves of 512 spatially-coherent queries; each
leaf only visits the <=ceil(pts/128) grid chunks covering its bounding box
+ margin. Every chunk is SELF-CONTAINED: it carries its own 128 grid
points (pa, fp16 hi/lo quadratic expansion, K=15) plus a copy of its
leaf's 512 samples (xa), so chunks are freely load-balanced across the 8
cores (T = ceil(total_chunks/8) per core) and multi-chunk leaves simply
produce partial sums that the host adds back together. Per chunk:
  mm1: exponent[128, 512] = pa^T @ xa  (PE, accumulate fp32 in PSUM)
  exp: ScalarE activation PSUM->SBUF fp16 (chunk pairs share one instr;
       the final pair runs as two singles so the last mm2 starts earlier)
  mm2: py[64, 512] = ct_chunk^T @ W    (PE; ct columns m = d*20+k hold
       [mask*adj_x | mask*adj_y | mask] per time bin k)
Chunk pairs share one [128, 512] PSUM bank for py (even chunk partitions
0:64, odd 64:128); the fp32->fp16 convert runs on the otherwise-idle
Vector engine (the last pair on ScalarE, idle after the final exp, so the
two tail converts run in parallel) and streams out as fp16. Host
epilogue: accumulate per-leaf partial num/den, per-sample time-bin
select, -num/(den+eps) divide (O(B) numpy).

All 8 cores run the identical program (T baked at compile); cores with
fewer real chunks get zero-padded dummy chunks.
"""
import numpy as np

import concourse.bass as bass
import concourse.tile as tile
from concourse import mybir, bacc
from concourse.bass_utils import run_bass_kernel_spmd

F32 = mybir.dt.float32
FP16 = mybir.dt.float16
FP16_NP = np.float16

B = 32768
GSIZE = 50
NBINS = 20
NCORES = 8
BG = 256           # samples per leaf
EPS = 1e-10
MARGIN = 1.4       # neighborhood radius: truncation rel err ~4e-3
PAD_EXP = -60000.0  # fp16-representable; exp() -> 0
CB_CLAMP = 20.0    # max per-sample exponent normalization
CW = 128 + BG      # px columns per chunk: [pa(128) | xa(BG)]
CPT = 4            # chunks per py/output tile

_CACHE = {}


def _build_nc(T):
    NQ = -(-T // 4)            # exp quads
    NT = -(-T // CPT)          # output tiles
    nc = bacc.Bacc("TRN2", target_bir_lowering=False)
    px_d = nc.dram_tensor("px", [15, T * CW], FP16, kind="ExternalInput")
    ct_d = nc.dram_tensor("ct", [128, T * 64], FP16, kind="ExternalInput")
    o_d = nc.dram_tensor("o", [NT, 128, 2 * BG], FP16, kind="ExternalOutput")

    with tile.TileContext(nc) as tc:
        with (
            tc.tile_pool(name="consts", bufs=1) as consts,
            tc.tile_pool(name="wt", bufs=6) as wtp,
            tc.tile_pool(name="pw", bufs=3, space="PSUM") as pwp,
            tc.tile_pool(name="py", bufs=2, space="PSUM") as pyp,
        ):
            # Head DMA covers the first two exp-quads (8 chunks) so the PE
            # never waits on the body stream; the rest follows in one DMA.
            # ct goes on a second engine queue so transfers run in parallel.
            px_sb = consts.tile([15, T * CW], FP16)
            HEAD = min(8, T) * CW
            nc.sync.dma_start(out=px_sb[:, 0:HEAD], in_=px_d[:, 0:HEAD])
            ct_sb = consts.tile([128, T * 64], FP16)
            if HEAD < T * CW:
                nc.sync.dma_start(out=px_sb[:, HEAD:], in_=px_d[:, HEAD:])
            nc.gpsimd.dma_start(out=ct_sb[:], in_=ct_d[:, :])
            out_sb = consts.tile([128, NT, 2 * BG], FP16)

            py_tiles = [None] * NT
            pend = []

            def emit_mm2(c, wt):
                p, r = c // CPT, c % CPT
                if r == 0:
                    py_tiles[p] = pyp.tile([128, 2, BG], F32, name="py")
                nc.tensor.matmul(
                    py_tiles[p][(r % 2) * 64:(r % 2) * 64 + 64, r // 2, :],
                    lhsT=ct_sb[:, c * 64:(c + 1) * 64],
                    rhs=wt[:], start=True, stop=True)
                if c == T - 1 or r == CPT - 1:
                    # tile complete: fp32 PSUM -> fp16 SBUF convert, then
                    # out. The last tile converts on ScalarE (idle after the
                    # final exp) so the two tail converts run in parallel.
                    if p == NT - 1:
                        nc.scalar.copy(out_sb[:, p, :], py_tiles[p][:, :, :])
                    else:
                        nc.vector.tensor_scalar_mul(out_sb[:, p, :],
                                                    py_tiles[p][:, :, :], 1.0)
                    nc.sync.dma_start(out=o_d[p], in_=out_sb[:, p, :])

            # Exp group sizes: the ragged group runs FIRST so the ACT
            # chain starts after only T%4 mm1s and every later group is a
            # full quad -- the chain both starts and ends earlier than a
            # quads-then-remainder order.
            sizes = ([T % 4] if T % 4 else []) + [4] * (T // 4)
            c0 = 0
            for w in sizes:
                pw = pwp.tile([128, 4, BG], F32, name="pw")
                for j in range(w):
                    c = c0 + j
                    nc.tensor.matmul(
                        pw[:, j, :], lhsT=px_sb[:, c * CW:c * CW + 128],
                        rhs=px_sb[:, c * CW + 128:(c + 1) * CW],
                        start=True, stop=True)
                wt = wtp.tile([128, 4, BG], FP16, name="wt")
                nc.scalar.activation(wt[:, 0:w, :], pw[:, 0:w, :],
                                     mybir.ActivationFunctionType.Exp)
                for j in range(w):
                    pend.append((c0 + j, wt[:, j, :]))
                c0 += w
                while len(pend) > 8:
                    cp, wslice = pend.pop(0)
                    emit_mm2(cp, wslice)
            for cp, wslice in pend:
                emit_mm2(cp, wslice)
    nc.compile()
    return nc


def _split_leaves(x):
    """Longest-axis k-d median split into leaves of BG sample indices."""
    leaves = []

    def rec(idx):
        if len(idx) == BG:
            leaves.append(idx)
            return
        xc = np.clip(x[idx], -8.3, 8.3)
        ax = int(np.argmax(xc.max(0) - xc.min(0)))
        order = np.argsort(x[idx, ax], kind="stable")
        h = len(idx) // 2
        rec(idx[order[:h]])
        rec(idx[order[h:]])

    rec(np.arange(x.shape[0]))
    return leaves


def _hi_lo(v):
    hi = v.astype(FP16_NP)
    lo = (v - hi.astype(np.float32)).astype(FP16_NP)
    return hi, lo


def kernel(t, x, grid_points, grid_adjoints, t_edges, grid_counts,
           trace=False, tmpdir=None):
    t = np.asarray(t, np.float32).reshape(B)
    x = np.asarray(x, np.float32)
    gp = np.asarray(grid_points, np.float32)
    adj = np.asarray(grid_adjoints, np.float32)
    te = np.asarray(t_edges, np.float32)
    cnt = np.asarray(grid_counts)
    G = gp.shape[0]

    tb = np.clip(np.searchsorted(te[1:-1], t, side="left"), 0, NBINS - 1)
    lin = gp[:GSIZE, 1]  # linspace(-8, 8, 50): y varies fastest (ij indexing)
    h = float(lin[1] - lin[0])

    # Per-sample exponent normalization c_b = min(2*d^2(nearest grid pt), 20):
    # keeps each sample's max weight near 1 so fp16 W never underflows for
    # spatial outliers. num/den both scale by exp(c_b); the host divide uses
    # eps*exp(c_b) so the result is exactly the reference ratio.
    gnear = np.clip(np.round((x - lin[0]) / h), 0, GSIZE - 1) * h + lin[0]
    cb = np.minimum(2.0 * ((x - gnear) ** 2).sum(1), CB_CLAMP)
    c16 = cb.astype(FP16_NP).astype(np.float32)

    leaves = _split_leaves(x)
    NL = len(leaves)

    # per-leaf grid neighborhood (index box), split into <=128-pt chunks
    chunks = []  # (leaf_id, grid_idx_array)
    for li, idx in enumerate(leaves):
        lo = x[idx].min(0) - MARGIN
        hi = x[idx].max(0) + MARGIN
        i0 = int(np.clip(np.searchsorted(lin, lo[0], "left"), 0, GSIZE - 1))
        i1 = int(np.clip(np.searchsorted(lin, hi[0], "right"), i0 + 1, GSIZE))
        j0 = int(np.clip(np.searchsorted(lin, lo[1], "left"), 0, GSIZE - 1))
        j1 = int(np.clip(np.searchsorted(lin, hi[1], "right"), j0 + 1, GSIZE))
        ii, jj = np.meshgrid(np.arange(i0, i1), np.arange(j0, j1),
                             indexing="ij")
        gidx = (ii * GSIZE + jj).reshape(-1)
        for c0 in range(0, len(gidx), 128):
            chunks.append((li, gidx[c0:c0 + 128]))

    T = -(-len(chunks) // NCORES)
    # round-robin: core k gets chunks k, k+8, ... (balanced within 1)
    percore = [[chunks[i] for i in range(k, len(chunks), NCORES)]
               for k in range(NCORES)]

    # precompute full-grid quadratic expansion (f32) and ct rows (f32)
    p5 = np.empty((5, G), np.float32)
    p5[0] = 4.0 * gp[:, 0]
    p5[1] = 4.0 * gp[:, 1]
    p5[2] = -2.0
    p5[3] = -2.0
    p5[4] = -2.0 * (gp[:, 0] ** 2 + gp[:, 1] ** 2)
    mask = (cnt > 0).astype(np.float32)                 # (20, G)
    ct_full = np.empty((G, 64), np.float32)
    ct_full[:, 0:20] = (mask * adj[:, :, 0]).T
    ct_full[:, 20:40] = (mask * adj[:, :, 1]).T
    ct_full[:, 40:60] = mask.T
    ct_full[:, 60:64] = 0.0

    # per-leaf xa block (built once, replicated into each of its chunks)
    xa_leaf = np.zeros((NL, 15, BG), FP16_NP)
    for li, idx in enumerate(leaves):
        xs = x[idx]
        xa = np.zeros((15, BG), np.float32)
        x_hi, x_lo = _hi_lo(xs.T)                   # (2, BG)
        sq_hi, sq_lo = _hi_lo(xs.T.astype(np.float32) ** 2)
        xa[0:2] = x_hi
        xa[2:4] = sq_hi
        xa[4] = 1.0
        xa[5:7] = x_lo
        xa[7:9] = sq_lo
        xa[9] = c16[idx]
        xa[10:12] = x_hi
        xa[12:14] = sq_hi
        xa[14] = 1.0
        xa_leaf[li] = xa.astype(FP16_NP)

    in_maps = []
    for k in range(NCORES):
        px = np.zeros((15, T * CW), FP16_NP)
        ct = np.zeros((T * 128, 64), FP16_NP)
        for ci in range(T):
            col = ci * CW
            if ci < len(percore[k]):
                li, gidx = percore[k][ci]
                n = len(gidx)
                pa = np.zeros((15, 128), FP16_NP)
                pa[4] = PAD_EXP
                pa[9] = 1.0
                p_hi, p_lo = _hi_lo(p5[:, gidx])        # (5, n)
                pa[0:5, 0:n] = p_hi
                pa[5:9, 0:n] = p_hi[0:4]
                pa[10:15, 0:n] = p_lo
                px[:, col:col + 128] = pa
                px[:, col + 128:col + CW] = xa_leaf[li]
                ct[128 * ci:128 * ci + n] = ct_full[gidx]
            else:
                px[4, col:col + 128] = PAD_EXP          # dummy: exp -> 0
        ct_dram = np.ascontiguousarray(
            ct.reshape(T, 128, 64).transpose(1, 0, 2).reshape(128, T * 64))
        in_maps.append({"px": px, "ct": ct_dram})

    key = ("nc", T)
    if key not in _CACHE:
        _CACHE[key] = _build_nc(T)
    nc = _CACHE[key]
    res = run_bass_kernel_spmd(nc, in_maps, core_ids=list(range(NCORES)),
                               trace=trace, tmpdir=tmpdir)
    _CACHE["last_result"] = res

    # host epilogue: accumulate per-leaf partials, select time bin, divide
    acc = np.zeros((NL, 64, BG), np.float32)
    for k in range(NCORES):
        o = np.asarray(res.results[k]["o"]).astype(np.float32)  # (NT,128,2BG)
        for ci, (li, _) in enumerate(percore[k]):
            p, r = ci // CPT, ci % CPT
            acc[li] += o[p, (r % 2) * 64:(r % 2) * 64 + 64,
                         (r // 2) * BG:(r // 2) * BG + BG]

    out = np.empty((B, 2), np.float32)
    jcol = np.arange(BG)
    for li, idx in enumerate(leaves):
        blk = acc[li]
        k = tb[idx]
        den = blk[40 + k, jcol] + EPS * np.exp(c16[idx])
        out[idx, 0] = -blk[k, jcol] / den
        out[idx, 1] = -blk[20 + k, jcol] / den
    return out
